# revision 1
# baseline (speedup 1.0000x reference)
"""Trainium2 Bass kernel for nn_CrossAttentionBlock.

Per-core work (data-parallel over batch, core b handles batch element b):
  q = avgpool2(query); k = avgpool2(kv)                 (pool scale folded into weights)
  Q = Wq' @ q, K = Wk' @ k   ([o, s] layout, attn scale folded into Wq')
  V_T = k^T @ Wv'^T          ([s, o] layout -- produced pre-transposed)
  per head: S_T = K_h^T Q_h  ([k, q] layout), expS = exp(S_T) (no max-sub; scores are O(1))
            O_T = V_h^T^T expS (PE, accumulated over k-tiles)
            rowsum via ones-matmul (replicated across 32 partitions)
            O_n = O_T * 1/rowsum
  Y = Wo' @ O_n              (BN gamma/var + upsample 1/16 folded into Wo')
  out = upsample2x_bilinear(Y) + g*query + b   (separable 2x (3,1)/4 taps)

Attention matmuls and projections run in bf16 (PSUM accumulation stays fp32);
identity/upsample/output path stays fp32. Self-contained: hardcodes shapes;
host-side numpy only folds the 256x256 weight matrices.
"""

import os
import sys

sys.path.insert(0, "/opt/trn_rl_repo")

import numpy as np
import ml_dtypes

import concourse.bass as bass
import concourse.tile as tile
from concourse import bacc, mybir
from concourse.bass_utils import run_bass_kernel_spmd

F32 = mybir.dt.float32
BF16 = mybir.dt.bfloat16
EPS = 1e-5

C = 256          # channels
HW = 4096        # 64*64
S = 1024         # pooled spatial 32*32
NCORES = 8
KT = 8           # k tiles of 128 over S
G = 2            # head groups (4 heads each) == channel tiles


def emit_kernel(tc, dram, stages=99):
    nc = tc.nc
    from contextlib import ExitStack

    query_d, kv_d = dram["query"], dram["kv"]
    w_d = {n: dram[n] for n in ("wqt", "wkt", "wvt", "wot")}
    gvec_d, bvec_d = dram["gvec"], dram["bvec"]
    out_d = dram["out"]

    with ExitStack() as ctx:
        consts = ctx.enter_context(tc.tile_pool(name="consts", bufs=1))
        wsb = {}
        for name, dt in (("wqt", BF16), ("wkt", BF16), ("wvt", BF16), ("wot", F32)):
            tiles = []
            for g in range(2):
                t = consts.tile([128, 256], dt, tag=f"w_{name}_{g}")
                nc.sync.dma_start(out=t[:], in_=w_d[name][g * 128:(g + 1) * 128, :])
                tiles.append(t)
            wsb[name] = tiles
        g_sb, b_sb = [], []
        for m in range(2):
            tg = consts.tile([128, 1], F32, tag=f"gv_{m}")
            nc.sync.dma_start(out=tg[:], in_=gvec_d[m * 128:(m + 1) * 128, :])
            g_sb.append(tg)
            tb = consts.tile([128, 1], F32, tag=f"bv_{m}")
            nc.sync.dma_start(out=tb[:], in_=bvec_d[m * 128:(m + 1) * 128, :])
            b_sb.append(tb)
        ones32 = consts.tile([128, 32], BF16, tag="ones32")
        nc.vector.memset(ones32[:], 1.0)

        # ---------------- input load + 2x2 sum-pool ----------------
        # query: full c-tiles, kept resident (identity path reads it at the end)
        # kv: loaded in spatial halves so attention can start after the top half
        qres = ctx.enter_context(tc.tile_pool(name="qres", bufs=1))
        kvbuf = ctx.enter_context(tc.tile_pool(name="kvbuf", bufs=2))
        poolw = ctx.enter_context(tc.tile_pool(name="poolw", bufs=2))
        pools = ctx.enter_context(tc.tile_pool(name="pools", bufs=1))

        q_tiles = []
        for g in range(2):
            t = qres.tile([128, HW], F32, tag=f"qres{g}")
            nc.sync.dma_start(out=t[:], in_=query_d[g * 128:(g + 1) * 128, :])
            q_tiles.append(t)

        def pool_half(raw, dst_pool_tile, rows):
            # raw: [128, 2048] fp32 = 32 spatial rows (64 wide); rows: out rows (16)
            rawv = raw.rearrange("p (h w t) -> p h w t", h=32, w=32, t=2)
            pw = poolw.tile([128, 1024], BF16, tag="pw")
            pwv = pw[:].rearrange("p (h w) -> p h w", h=32)
            nc.vector.tensor_add(pwv, rawv[:, :, :, 0], rawv[:, :, :, 1])
            pw2 = pw[:].rearrange("p (h t w) -> p h t w", h=16, t=2, w=32)
            nc.gpsimd.tensor_add(dst_pool_tile, pw2[:, :, 0, :], pw2[:, :, 1, :])

        # query pools (full, from resident tiles)
        q_pool = []
        for g in range(2):
            qp = pools.tile([128, S], BF16, tag=f"qpool{g}")
            qpv = qp[:].rearrange("p (h w) -> p h w", h=32)
            for half in range(2):
                pool_half(q_tiles[g][:, half * 2048:(half + 1) * 2048],
                          qpv[:, half * 16:(half + 1) * 16, :], 16)
            q_pool.append(qp)

        # kv pools, per spatial half
        k_pool = []
        for g in range(2):
            kp = pools.tile([128, S], BF16, tag=f"kpool{g}")
            k_pool.append(kp)
        for half in range(2):
            for g in range(2):
                raw = kvbuf.tile([128, 2048], F32, tag="kvraw")
                nc.sync.dma_start(
                    out=raw[:],
                    in_=kv_d[g * 128:(g + 1) * 128, half * 2048:(half + 1) * 2048])
                kpv = k_pool[g][:].rearrange("p (h w) -> p h w", h=32)
                pool_half(raw[:], kpv[:, half * 16:(half + 1) * 16, :], 16)

        if stages < 1:
            nc.sync.dma_start(out=out_d[0:128, 0:512],
                              in_=q_pool[0][:].bitcast(F32))
            return

        # ---------------- projections (bf16 in, fp32 psum) ----------------
        qk_sb = ctx.enter_context(tc.tile_pool(name="qk_sb", bufs=1))
        vt_sb_pool = ctx.enter_context(tc.tile_pool(name="vt_sb", bufs=1))
        Q_sb = {}   # (m, qh) -> [128, 512] bf16
        K_sb = {}   # (m, sh) -> [128, 512] bf16
        vt_sb = []

        with tc.tile_pool(name="psA", bufs=2, space="PSUM") as psA, \
             tc.tile_pool(name="psV", bufs=2, space="PSUM") as psV:

            def proj_qk(dst, wname, src, m):
                pt = psA.tile([128, 1024], F32, tag="qk")
                for nh in range(2):
                    for g in range(2):
                        nc.tensor.matmul(
                            pt[:, nh * 512:(nh + 1) * 512],
                            lhsT=wsb[wname][g][:, m * 128:(m + 1) * 128],
                            rhs=src[g][:, nh * 512:(nh + 1) * 512],
                            start=(g == 0), stop=(g == 1),
                        )
                    st = qk_sb.tile([128, 512], BF16, tag=f"{wname}_{m}_{nh}")
                    nc.vector.tensor_copy(st[:], pt[:, nh * 512:(nh + 1) * 512])
                    dst[(m, nh)] = st

            def proj_vt(b):
                pt = psV.tile([128, 256], F32, tag="vt")
                for g in range(2):
                    nc.tensor.matmul(
                        pt[:],
                        lhsT=k_pool[g][:, b * 128:(b + 1) * 128],
                        rhs=wsb["wvt"][g][:],
                        start=(g == 0), stop=(g == 1),
                    )
                st = vt_sb_pool.tile([128, 256], BF16, tag=f"vt{b}")
                nc.vector.tensor_copy(st[:], pt[:])
                vt_sb.append(st)

            proj_qk(Q_sb, "wqt", q_pool, 0)
            proj_qk(K_sb, "wkt", k_pool, 0)
            for b in range(KT):
                proj_vt(b)
            proj_qk(Q_sb, "wqt", q_pool, 1)
            proj_qk(K_sb, "wkt", k_pool, 1)

        if stages < 2:
            nc.sync.dma_start(out=out_d[0:128, 0:256],
                              in_=Q_sb[(0, 0)][:].bitcast(F32))
            return

        # ---------------- attention + output ----------------
        expp = ctx.enter_context(tc.tile_pool(name="expp", bufs=4))
        rcpp = ctx.enter_context(tc.tile_pool(name="rcpp", bufs=2))
        onp = ctx.enter_context(tc.tile_pool(name="onp", bufs=4))
        ysbp = ctx.enter_context(tc.tile_pool(name="ysbp", bufs=1))
        y3p = ctx.enter_context(tc.tile_pool(name="y3p", bufs=1))
        tup = ctx.enter_context(tc.tile_pool(name="tup", bufs=1))
        t3p = ctx.enter_context(tc.tile_pool(name="t3p", bufs=2))
        affp = ctx.enter_context(tc.tile_pool(name="affp", bufs=1))
        finp = ctx.enter_context(tc.tile_pool(name="finp", bufs=2))

        on_t = {}    # (g, qh) -> [128, 512] normalized attention out
        ysb = {}     # (m, qh) -> [128, 512] Y (pooled, scaled)
        Tt = {}      # (m, half) -> [128, 1024] W-upsampled rows
        aff = {}     # (m, half) -> [128, 2048] g*identity + b

        with tc.tile_pool(name="psS", bufs=2, space="PSUM") as psS, \
             tc.tile_pool(name="psPV", bufs=1, space="PSUM") as psPV, \
             tc.tile_pool(name="psRSY", bufs=2, space="PSUM") as psRSY:

            def attn_chunk(g, qh):
                OT = psPV.tile([128, 512], F32, tag="ot")
                RS = psRSY.tile([128, 512], F32, tag="rsy")
                for b in range(KT):
                    for sub in range(2):      # head pairs (0,1) / (2,3)
                        Sp = psS.tile([128, 1024], F32, tag="sc")
                        for jj in range(2):
                            j = 2 * sub + jj
                            nc.tensor.matmul(
                                Sp[:, jj * 512:(jj + 1) * 512],
                                lhsT=K_sb[(g, b // 4)][32 * j:32 * j + 32,
                                                       (b % 4) * 128:(b % 4) * 128 + 128],
                                rhs=Q_sb[(g, qh)][32 * j:32 * j + 32, :],
                                start=True, stop=True,
                                tile_position=(32 * j, 0),
                            )
                        eb = expp.tile([128, 1024], BF16, tag="exp")
                        nc.scalar.activation(eb[:], Sp[:],
                                             mybir.ActivationFunctionType.Exp)
                        for jj in range(2):
                            j = 2 * sub + jj
                            h = 4 * g + j
                            nc.tensor.matmul(
                                OT[32 * j:32 * j + 32, :],
                                lhsT=vt_sb[b][:, 32 * h:32 * h + 32],
                                rhs=eb[:, jj * 512:(jj + 1) * 512],
                                start=(b == 0), stop=(b == KT - 1),
                                tile_position=(0, 32 * j),
                                skip_group_check=True,
                            )
                            nc.tensor.matmul(
                                RS[32 * j:32 * j + 32, :],
                                lhsT=ones32[:],
                                rhs=eb[:, jj * 512:(jj + 1) * 512],
                                start=(b == 0), stop=(b == KT - 1),
                                tile_position=(0, 32 * j),
                                skip_group_check=True,
                            )
                rcp = rcpp.tile([128, 512], F32, tag="rcp")
                nc.vector.reciprocal_approx_fast(out=rcp[:], in_=RS[:])
                ot = onp.tile([128, 512], F32, tag="on")
                nc.vector.tensor_mul(ot[:], OT[:], rcp[:])
                on_t[(g, qh)] = ot

            def wo_proj(qh):
                for m in range(2):
                    yp = psRSY.tile([128, 512], F32, tag="rsy")
                    for g in range(2):
                        nc.tensor.matmul(
                            yp[:],
                            lhsT=wsb["wot"][g][:, m * 128:(m + 1) * 128],
                            rhs=on_t[(g, qh)][:],
                            start=(g == 0), stop=(g == 1),
                        )
                    st = ysbp.tile([128, 512], F32, tag=f"ysb{m}{qh}")
                    nc.vector.tensor_copy(st[:], yp[:])
                    ysb[(m, qh)] = st

            def make_aff(m, half):
                a = affp.tile([128, 2048], F32, tag=f"aff{m}{half}")
                nc.vector.tensor_scalar(
                    a[:], q_tiles[m][:, half * 2048:(half + 1) * 2048],
                    g_sb[m][:], b_sb[m][:],
                    op0=mybir.AluOpType.mult, op1=mybir.AluOpType.add,
                )
                aff[(m, half)] = a

            def w_upsample(m, half):
                y = ysb[(m, half)][:].rearrange("p (h w) -> p h w", h=16)
                y3t = y3p.tile([128, 512], F32, tag="y3")
                nc.vector.tensor_scalar_mul(y3t[:], ysb[(m, half)][:], 3.0)
                y3 = y3t[:].rearrange("p (h w) -> p h w", h=16)
                tt = tup.tile([128, 1024], F32, tag=f"t{m}{half}")
                t4 = tt[:].rearrange("p (h w t) -> p h w t", h=16, w=32, t=2)
                nc.vector.tensor_add(t4[:, :, 1:32, 0], y3[:, :, 1:32], y[:, :, 0:31])
                nc.vector.tensor_scalar_mul(t4[:, :, 0, 0], y[:, :, 0], 4.0)
                nc.vector.tensor_add(t4[:, :, 0:31, 1], y3[:, :, 0:31], y[:, :, 1:32])
                nc.vector.tensor_scalar_mul(t4[:, :, 31, 1], y[:, :, 31], 4.0)
                Tt[(m, half)] = tt

            def h_upsample_and_out(m, half):
                tc_t = Tt[(m, half)][:].rearrange("p (h x) -> p h x", h=16)
                t3t = t3p.tile([128, 1024], F32, tag="t3")
                nc.vector.tensor_scalar_mul(t3t[:], Tt[(m, half)][:], 3.0)
                t3 = t3t[:].rearrange("p (h x) -> p h x", h=16)
                fin = finp.tile([128, 2048], F32, tag="fin")
                f4 = fin[:].rearrange("p (h t x) -> p h t x", h=16, t=2, x=64)
                if half == 0:
                    nc.vector.tensor_scalar_mul(f4[:, 0, 0, :], tc_t[:, 0, :], 4.0)
                    nc.vector.tensor_add(f4[:, 1:16, 0, :], t3[:, 1:16, :], tc_t[:, 0:15, :])
                    nc.vector.tensor_add(f4[:, 0:15, 1, :], t3[:, 0:15, :], tc_t[:, 1:16, :])
                    tb = Tt[(m, 1)][:].rearrange("p (h x) -> p h x", h=16)
                    nc.vector.tensor_add(f4[:, 15, 1, :], t3[:, 15, :], tb[:, 0, :])
                else:
                    ttop = Tt[(m, 0)][:].rearrange("p (h x) -> p h x", h=16)
                    nc.vector.tensor_add(f4[:, 0, 0, :], t3[:, 0, :], ttop[:, 15, :])
                    nc.vector.tensor_add(f4[:, 1:16, 0, :], t3[:, 1:16, :], tc_t[:, 0:15, :])
                    nc.vector.tensor_add(f4[:, 0:15, 1, :], t3[:, 0:15, :], tc_t[:, 1:16, :])
                    nc.vector.tensor_scalar_mul(f4[:, 15, 1, :], tc_t[:, 15, :], 4.0)
                nc.gpsimd.tensor_add(fin[:], fin[:], aff[(m, half)][:])
                nc.sync.dma_start(
                    out=out_d[m * 128:(m + 1) * 128,
                              half * 2048:(half + 1) * 2048],
                    in_=fin[:],
                )

            # ---- pipeline: q-half 0, then q-half 1, tail ----
            attn_chunk(0, 0)
            if stages < 3:
                nc.sync.dma_start(out=out_d[0:128, 0:512], in_=on_t[(0, 0)][:])
                return
            attn_chunk(1, 0)
            wo_proj(0)
            make_aff(0, 0)
            make_aff(1, 0)
            w_upsample(0, 0)
            w_upsample(1, 0)
            attn_chunk(0, 1)
            make_aff(0, 1)
            make_aff(1, 1)
            attn_chunk(1, 1)
            wo_proj(1)
            if stages < 4:
                nc.sync.dma_start(out=out_d[0:128, 0:512], in_=ysb[(0, 0)][:])
                return
            w_upsample(0, 1)
            w_upsample(1, 1)
            for m in range(2):
                for half in range(2):
                    h_upsample_and_out(m, half)


def build_module(n_iters=1):
    nc = bacc.Bacc(
        "TRN2",
        target_bir_lowering=False,
        debug=False,
        enable_asserts=False,
    )
    dram = {}
    dram["query"] = nc.dram_tensor("query", [C, HW], F32, kind="ExternalInput").ap()
    dram["kv"] = nc.dram_tensor("kv", [C, HW], F32, kind="ExternalInput").ap()
    for n, dt in (("wqt", BF16), ("wkt", BF16), ("wvt", BF16), ("wot", F32)):
        dram[n] = nc.dram_tensor(n, [C, C], dt, kind="ExternalInput").ap()
    dram["gvec"] = nc.dram_tensor("gvec", [C, 1], F32, kind="ExternalInput").ap()
    dram["bvec"] = nc.dram_tensor("bvec", [C, 1], F32, kind="ExternalInput").ap()
    dram["out"] = nc.dram_tensor("out", [C, HW], F32, kind="ExternalOutput").ap()

    with tile.TileContext(nc) as tc:
        if n_iters == 1:
            emit_kernel(tc, dram)
        else:
            with tc.For_i(0, n_iters, 1):
                emit_kernel(tc, dram)
    nc.compile()
    return nc


_NC_CACHE = {}


def _get_module(n_iters=1):
    if n_iters not in _NC_CACHE:
        _NC_CACHE[n_iters] = build_module(n_iters)
    return _NC_CACHE[n_iters]


def fold_weights(Wq, Wk, Wv, Wo, bn_gamma, bn_beta, bn_mean, bn_var, num_heads):
    nh = int(num_heads)
    hd = C // nh
    scale = np.float32(hd ** -0.5)
    wqt = np.ascontiguousarray((0.25 * scale * Wq).T.astype(ml_dtypes.bfloat16))
    wkt = np.ascontiguousarray((0.25 * Wk).T.astype(ml_dtypes.bfloat16))
    wvt = np.ascontiguousarray((0.25 * Wv).T.astype(ml_dtypes.bfloat16))
    inv = 1.0 / np.sqrt(bn_var.astype(np.float32) + EPS)
    g = (bn_gamma * inv).astype(np.float32)
    bb = (bn_beta - bn_mean * bn_gamma * inv).astype(np.float32)
    wot = np.ascontiguousarray(((g[:, None] * Wo) / 16.0).T.astype(np.float32))
    return wqt, wkt, wvt, wot, g, bb


LAST_RESULTS = None


def kernel(query, kv, Wq, Wk, Wv, Wo, bn_gamma, bn_beta, bn_mean, bn_var, num_heads):
    global LAST_RESULTS
    query = np.asarray(query, dtype=np.float32)
    kv = np.asarray(kv, dtype=np.float32)
    assert int(num_heads) == 8 and query.shape == (NCORES, C, 64, 64)

    wqt, wkt, wvt, wot, g, bb = fold_weights(
        np.asarray(Wq, np.float32), np.asarray(Wk, np.float32),
        np.asarray(Wv, np.float32), np.asarray(Wo, np.float32),
        np.asarray(bn_gamma, np.float32), np.asarray(bn_beta, np.float32),
        np.asarray(bn_mean, np.float32), np.asarray(bn_var, np.float32),
        num_heads,
    )
    shared = {
        "wqt": wqt, "wkt": wkt, "wvt": wvt, "wot": wot,
        "gvec": np.ascontiguousarray(g.reshape(C, 1)),
        "bvec": np.ascontiguousarray(bb.reshape(C, 1)),
    }
    in_maps = []
    for b in range(NCORES):
        m = dict(shared)
        m["query"] = np.ascontiguousarray(query[b].reshape(C, HW))
        m["kv"] = np.ascontiguousarray(kv[b].reshape(C, HW))
        in_maps.append(m)

    nc = _get_module(int(os.environ.get("KERNEL_ITERS", "1")))
    res = run_bass_kernel_spmd(nc, in_maps, list(range(NCORES)))
    LAST_RESULTS = res
    out = np.stack([res.results[b]["out"].reshape(C, 64, 64) for b in range(NCORES)])
    return out.astype(np.float32)

